# Initial kernel scaffold
#
"""Trainium2 Bass kernel for DiamondLayer.

Computes out[b, d] = mean(x[b, d:d+16, d+17:d+33]) for d in [0, 2016):
16x16 mean-pool windows sliding along the diagonal of each 2048x2048 matrix.

Sharding: pure data parallel over batch - 32 batches -> 8 cores x 4 batches.

Per-core kernel (raw bacc, no Tile):
  - Only the diagonal band cols [r+2, r+34) of row r is ever touched, so each
    core DMAs just that band out of its full input shard with a strided access
    pattern: partition p holds rows [16p, 16p+16), each row one 128B run
    (2032 descriptors/batch instead of a 16MB matrix read; ~1/64 of the data).
  - VectorE computes a per-partition prefix scan P of the flat band
    (tensor_tensor_scan); window sums become differences of P.
  - An SBUF->SBUF DMA shifts each partition's P into the previous partition
    ("halo") at column 512, making S16[q,tau,m] = P[32tau+m+16] - P[32tau+m]
    one uniform strided subtract for all 31 window rows (done on GPSIMD).
  - A strided VectorE reduce sums 16 diagonal window terms per output:
    out[16q+u] = sum_s S16[q, u+s, 15-s]; GPSIMD scales by 1/256.
  - Engines: SP (band/halo/out DMAs), ACT (tail-partition DMAs),
    DVE (scan+reduce), Pool (sub+scale). Per-DMA semaphores (completion
    counts from one shared semaphore are not ordered across DMAs).
"""

import os
import sys

import numpy as np

for _p in ("/opt/trn_rl_repo",):
    if _p not in sys.path:
        sys.path.insert(0, _p)

B_FULL = 32
N_CORES = 8
B_PER_CORE = B_FULL // N_CORES  # 4
MAT = 2048
ND = MAT - 32  # 2016
NQ = ND // 16  # 126
NP = NQ + 1  # 127
ROW_STRIDE = MAT + 1  # 2049
MAT_ELEMS = MAT * MAT
PW = 1024

LAST_EXEC_TIME_NS = None
_COMPILED = None


def _build():
    import concourse.bass as bass
    import concourse.bacc as bacc
    from concourse import mybir
    from contextlib import ExitStack

    f32 = mybir.dt.float32
    add = mybir.AluOpType.add
    sub_op = mybir.AluOpType.subtract
    bypass = mybir.AluOpType.bypass
    X = mybir.AxisListType.X

    nc = bacc.Bacc("TRN2", target_bir_lowering=False, debug=False)
    x = nc.dram_tensor("x", [B_PER_CORE, MAT, MAT], f32, kind="ExternalInput")
    y = nc.dram_tensor("y", [B_PER_CORE, ND], f32, kind="ExternalOutput")

    def v(t, off, pat):
        return bass.AP(t, off, pat)

    with ExitStack() as ctx:
        B = B_PER_CORE
        e = ctx.enter_context
        bts = [e(nc.sbuf_tensor(f"bt{i}", [NP, PW], f32)) for i in range(B)]
        pps = [e(nc.sbuf_tensor(f"pp{i}", [NP, PW], f32)) for i in range(B)]
        ss = [e(nc.sbuf_tensor(f"s{i}", [NQ, 496], f32)) for i in range(B)]
        rs = [e(nc.sbuf_tensor(f"r{i}", [NQ, 16], f32)) for i in range(B)]
        ros = [e(nc.sbuf_tensor(f"ro{i}", [NQ, 16], f32)) for i in range(B)]
        bsem = [e(nc.semaphore(f"bsem{i}")) for i in range(B)]
        hsem = [e(nc.semaphore(f"hsem{i}")) for i in range(B)]
        tsems = [e(nc.semaphore(f"tsem{i}")) for i in range(B)]
        vscan = e(nc.semaphore("vscan"))
        pdone = e(nc.semaphore("pdone"))
        vchain = e(nc.semaphore("vchain"))
        vec_done = e(nc.semaphore("vec_done"))
        dma_out = e(nc.semaphore("dma_out"))
        block = e(nc.Block(no_gpsimd_drain=True))

        @block.scalar
        def _(scalar):
            # partition 126's band rows (halo source for q=125), all batches
            for b in range(B):
                scalar.dma_start(
                    v(bts[b], NQ * PW + 1, [[PW, 1], [32, 16], [1, 32]]),
                    bass.AP(
                        x,
                        b * MAT_ELEMS + 2 + NQ * 16 * ROW_STRIDE,
                        [[16 * ROW_STRIDE, 1], [ROW_STRIDE, 16], [1, 32]],
                    ),
                ).then_inc(tsems[b], 16)

        @block.sync
        def _(sync):
            for b in range(B):
                # band: bt[p, 1+32t+j] = x[b, 16p+t, 16p+t+2+j]
                sync.dma_start(
                    v(bts[b], 1, [[PW, NQ], [32, 16], [1, 32]]),
                    bass.AP(
                        x,
                        b * MAT_ELEMS + 2,
                        [[16 * ROW_STRIDE, NQ], [ROW_STRIDE, 16], [1, 32]],
                    ),
                ).then_inc(bsem[b], 16)
            for b in range(B):
                # halo: PPH[q, 512+g] = P[q+1, g], g in [0, 481)
                sync.wait_ge(vscan, b + 1)
                sync.dma_start(
                    v(pps[b], 512, [[PW, NQ], [1, 481]]),
                    v(pps[b], PW, [[PW, NQ], [1, 481]]),
                ).then_inc(hsem[b], 16)
            for b in range(B):
                sync.wait_ge(vec_done, b + 1)
                sync.dma_start(
                    bass.AP(y, b * ND, [[16, NQ], [1, 16]]),
                    v(ros[b], 0, [[16, NQ], [1, 16]]),
                ).then_inc(dma_out, 16)
            sync.wait_ge(dma_out, 16 * B)

        @block.vector
        def _(vector):
            for pp in pps:
                nc.vector.memset(pp[0:NP, 0:1], 0.0)
            for b in range(B):
                bt, pp = bts[b], pps[b]
                vector.wait_ge(bsem[b], 16)
                vector.wait_ge(tsems[b], 16)
                # P[f] = prefix sum of the flat band per partition; P[0] = 0
                nc.vector.tensor_tensor_scan(
                    out=v(pp, 1, [[PW, NP], [1, 511]]),
                    data0=v(bt, 1, [[PW, NP], [1, 511]]),
                    data1=v(bt, 1, [[PW, NP], [1, 511]]),
                    initial=0.0,
                    op0=add,
                    op1=bypass,
                ).then_inc(vscan, 1)
            for b in range(B):
                # out[16q+u] = sum_s S16[q, u+s, 15-s] (flat 16u+15s+15)
                vector.wait_ge(pdone, b + 1)
                nc.vector.reduce_sum(
                    out=v(rs[b], 0, [[16, NQ], [1, 16]]),
                    in_=v(ss[b], 15, [[496, NQ], [16, 16], [15, 16]]),
                    axis=X,
                ).then_inc(vchain, 1)

        @block.gpsimd
        def _(gpsimd):
            for b in range(B):
                pp, s16 = pps[b], ss[b]
                # S16[q,tau,m] = PPH[32tau+m+16] - PPH[32tau+m], tau in [0,31)
                gpsimd.wait_ge(hsem[b], 16)
                nc.gpsimd.tensor_tensor(
                    out=v(s16, 0, [[496, NQ], [16, 31], [1, 16]]),
                    in0=v(pp, 16, [[PW, NQ], [32, 31], [1, 16]]),
                    in1=v(pp, 0, [[PW, NQ], [32, 31], [1, 16]]),
                    op=sub_op,
                ).then_inc(pdone, 1)
            for b in range(B):
                gpsimd.wait_ge(vchain, b + 1)
                nc.gpsimd.tensor_scalar_mul(
                    v(ros[b], 0, [[16, NQ], [1, 16]]),
                    v(rs[b], 0, [[16, NQ], [1, 16]]),
                    1.0 / 256.0,
                ).then_inc(vec_done, 1)

    nc.compile()
    return nc


def _get_compiled():
    global _COMPILED
    if _COMPILED is None:
        _COMPILED = _build()
    return _COMPILED


def kernel(x: np.ndarray) -> np.ndarray:
    global LAST_EXEC_TIME_NS
    from concourse.bass_utils import run_bass_kernel_spmd

    x = np.ascontiguousarray(np.asarray(x), dtype=np.float32)
    assert x.shape == (B_FULL, MAT, MAT), x.shape

    nc = _get_compiled()
    in_maps = [
        {"x": x[i * B_PER_CORE : (i + 1) * B_PER_CORE]} for i in range(N_CORES)
    ]
    trace = bool(int(os.environ.get("KERNEL_TRACE", "0")))
    if trace:
        # test-only: keep NTFF artifacts local instead of uploading
        from concourse import bass_utils as _bu

        _bu.upload_artifacts = lambda tmpdir: tmpdir
    res = run_bass_kernel_spmd(
        nc, in_maps, core_ids=list(range(N_CORES)), trace=trace
    )
    LAST_EXEC_TIME_NS = res.exec_time_ns
    out = np.concatenate([res.results[i]["y"] for i in range(N_CORES)], axis=0)
    return out.astype(np.float32)



# revision 1
# speedup vs baseline: 3.3467x; 3.3467x over previous
"""Trainium2 Bass kernel for DiamondLayer.

Computes out[b, d] = mean(x[b, d:d+16, d+17:d+33]) for d in [0, 2016):
16x16 mean-pool windows sliding along the diagonal of each 2048x2048 matrix.

Sharding: pure data parallel over batch - 32 batches -> 8 cores x 4 batches.

Per-core kernel (raw bacc, no Tile):
  - Only the diagonal band cols [r+2, r+34) of row r is ever touched, so each
    core DMAs just that band out of its full input shard with a strided access
    pattern: partition p holds rows [16p, 16p+16), each row one 128B run
    (2032 descriptors/batch instead of a 16MB matrix read; ~1/64 of the data).
  - VectorE computes a per-partition prefix scan P of the flat band
    (tensor_tensor_scan); window sums become differences of P.
  - An SBUF->SBUF DMA shifts each partition's P into the previous partition
    ("halo") at column 512, making S16[q,tau,m] = P[32tau+m+16] - P[32tau+m]
    one uniform strided subtract for all 31 window rows (done on GPSIMD).
  - A strided VectorE reduce sums 16 diagonal window terms per output:
    out[16q+u] = sum_s S16[q, u+s, 15-s]; GPSIMD scales by 1/256.
  - Engines: SP (band/halo/out DMAs), ACT (tail-partition DMAs),
    DVE (scan+reduce), Pool (sub+scale). Per-DMA semaphores (completion
    counts from one shared semaphore are not ordered across DMAs).
"""

import os
import sys

import numpy as np

for _p in ("/opt/trn_rl_repo",):
    if _p not in sys.path:
        sys.path.insert(0, _p)

B_FULL = 32
N_CORES = 8
B_PER_CORE = B_FULL // N_CORES  # 4
MAT = 2048
ND = MAT - 32  # 2016
NQ = ND // 16  # 126
NP = NQ + 1  # 127
ROW_STRIDE = MAT + 1  # 2049
MAT_ELEMS = MAT * MAT
PW = 1024

LAST_EXEC_TIME_NS = None
_COMPILED = None


def _build():
    import concourse.bass as bass
    import concourse.bacc as bacc
    from concourse import mybir
    from contextlib import ExitStack

    f32 = mybir.dt.float32
    add = mybir.AluOpType.add
    sub_op = mybir.AluOpType.subtract
    bypass = mybir.AluOpType.bypass
    X = mybir.AxisListType.X

    nc = bacc.Bacc("TRN2", target_bir_lowering=False, debug=False)
    x = nc.dram_tensor("x", [B_PER_CORE, MAT, MAT], f32, kind="ExternalInput")
    y = nc.dram_tensor("y", [B_PER_CORE, ND], f32, kind="ExternalOutput")

    def v(t, off, pat):
        return bass.AP(t, off, pat)

    with ExitStack() as ctx:
        B = B_PER_CORE
        e = ctx.enter_context
        bts = [e(nc.sbuf_tensor(f"bt{i}", [NP, PW], f32)) for i in range(B)]
        pps = [e(nc.sbuf_tensor(f"pp{i}", [NP, PW], f32)) for i in range(B)]
        ss = [e(nc.sbuf_tensor(f"s{i}", [NQ, 496], f32)) for i in range(B)]
        rs = [e(nc.sbuf_tensor(f"r{i}", [NQ, 16], f32)) for i in range(B)]
        ros = [e(nc.sbuf_tensor(f"ro{i}", [NQ, 16], f32)) for i in range(B)]
        bsem = [e(nc.semaphore(f"bsem{i}")) for i in range(B)]
        hsem = [e(nc.semaphore(f"hsem{i}")) for i in range(B)]
        tsems = [e(nc.semaphore(f"tsem{i}")) for i in range(B)]
        vscan = e(nc.semaphore("vscan"))
        pdone = e(nc.semaphore("pdone"))
        vchain = e(nc.semaphore("vchain"))
        vec_done = e(nc.semaphore("vec_done"))
        dma_out = e(nc.semaphore("dma_out"))
        block = e(nc.Block(no_gpsimd_drain=True))

        @block.scalar
        def _(scalar):
            # partition 126's band rows (halo source for q=125), all batches
            for b in range(B):
                scalar.dma_start(
                    v(bts[b], NQ * PW + 1, [[PW, 1], [32, 16], [1, 32]]),
                    bass.AP(
                        x,
                        b * MAT_ELEMS + 2 + NQ * 16 * ROW_STRIDE,
                        [[16 * ROW_STRIDE, 1], [ROW_STRIDE, 16], [1, 32]],
                    ),
                ).then_inc(tsems[b], 16)

        @block.sync
        def _(sync):
            for b in range(B):
                # band: bt[p, 1+32t+j] = x[b, 16p+t, 16p+t+2+j]
                sync.dma_start(
                    v(bts[b], 1, [[PW, NQ], [32, 16], [1, 32]]),
                    bass.AP(
                        x,
                        b * MAT_ELEMS + 2,
                        [[16 * ROW_STRIDE, NQ], [ROW_STRIDE, 16], [1, 32]],
                    ),
                ).then_inc(bsem[b], 16)
            for b in range(B):
                # halo: PPH[q, 512+g] = P[q+1, g], g in [0, 481)
                sync.wait_ge(vscan, b + 1)
                sync.dma_start(
                    v(pps[b], 512, [[PW, NQ], [1, 481]]),
                    v(pps[b], PW, [[PW, NQ], [1, 481]]),
                ).then_inc(hsem[b], 16)
            for b in range(B):
                sync.wait_ge(vec_done, b + 1)
                sync.dma_start(
                    bass.AP(y, b * ND, [[16, NQ], [1, 16]]),
                    v(ros[b], 0, [[16, NQ], [1, 16]]),
                ).then_inc(dma_out, 16)
            sync.wait_ge(dma_out, 16 * B)

        @block.vector
        def _(vector):
            for pp in pps:
                nc.vector.memset(pp[0:NP, 0:1], 0.0)
            for b in range(B):
                bt, pp = bts[b], pps[b]
                vector.wait_ge(bsem[b], 16)
                vector.wait_ge(tsems[b], 16)
                # P[f] = prefix sum of the flat band per partition; P[0] = 0
                nc.vector.tensor_tensor_scan(
                    out=v(pp, 1, [[PW, NP], [1, 511]]),
                    data0=v(bt, 1, [[PW, NP], [1, 511]]),
                    data1=v(bt, 1, [[PW, NP], [1, 511]]),
                    initial=0.0,
                    op0=add,
                    op1=bypass,
                ).then_inc(vscan, 1)
            for b in range(B):
                # out[16q+u] = sum_s S16[q, u+s, 15-s] (flat 16u+15s+15)
                vector.wait_ge(pdone, b + 1)
                nc.vector.reduce_sum(
                    out=v(rs[b], 0, [[16, NQ], [1, 16]]),
                    in_=v(ss[b], 15, [[496, NQ], [16, 16], [15, 16]]),
                    axis=X,
                ).then_inc(vchain, 1)

        @block.gpsimd
        def _(gpsimd):
            for b in range(B):
                pp, s16 = pps[b], ss[b]
                # S16[q,tau,m] = PPH[32tau+m+16] - PPH[32tau+m], tau in [0,31)
                gpsimd.wait_ge(hsem[b], 16)
                nc.gpsimd.tensor_tensor(
                    out=v(s16, 0, [[496, NQ], [16, 31], [1, 16]]),
                    in0=v(pp, 16, [[PW, NQ], [32, 31], [1, 16]]),
                    in1=v(pp, 0, [[PW, NQ], [32, 31], [1, 16]]),
                    op=sub_op,
                ).then_inc(pdone, 1)
            for b in range(B):
                gpsimd.wait_ge(vchain, b + 1)
                nc.gpsimd.tensor_scalar_mul(
                    v(ros[b], 0, [[16, NQ], [1, 16]]),
                    v(rs[b], 0, [[16, NQ], [1, 16]]),
                    1.0 / 256.0,
                ).then_inc(vec_done, 1)

    nc.compile()
    return nc


def _get_compiled():
    global _COMPILED
    if _COMPILED is None:
        _COMPILED = _build()
    return _COMPILED


def kernel(x: np.ndarray) -> np.ndarray:
    global LAST_EXEC_TIME_NS
    from concourse.bass_utils import run_bass_kernel_spmd

    x = np.ascontiguousarray(np.asarray(x), dtype=np.float32)
    assert x.shape == (B_FULL, MAT, MAT), x.shape

    nc = _get_compiled()
    in_maps = [
        {"x": x[i * B_PER_CORE : (i + 1) * B_PER_CORE]} for i in range(N_CORES)
    ]
    trace = bool(int(os.environ.get("KERNEL_TRACE", "0")))
    if trace:
        # test-only: keep NTFF artifacts local instead of uploading
        from concourse import bass_utils as _bu

        _bu.upload_artifacts = lambda tmpdir: tmpdir
    res = run_bass_kernel_spmd(
        nc, in_maps, core_ids=list(range(N_CORES)), trace=trace
    )
    LAST_EXEC_TIME_NS = res.exec_time_ns
    out = np.concatenate([res.results[i]["y"] for i in range(N_CORES)], axis=0)
    return out.astype(np.float32)

